# revision 17
# baseline (speedup 1.0000x reference)
"""2-layer GCN forward on 8 Trainium2 NeuronCores (Bass/Tile).

Strategy:
  out = dinv * (A_w @ u) + b   per layer, where u = dinv * (x @ W) and
  A_w is the weighted adjacency (incl. self loops w=1), dinv = rsqrt(deg).
  This removes per-edge norm computation entirely.

  - Nodes padded to NPAD = 8*NBLK*128; core i owns dest blocks
    [i*NBLK, (i+1)*NBLK).  Edges are sorted by dest block on host and
    padded into fixed-size chunks of 128 messages.
  - Gather side: dma_gather (SWDGE MoE primitive) pulls message rows
    u[src] from a DRAM table (256B rows).  int16 index reach handled by
    a lo/hi table split at 32768.
  - Scatter side: per chunk a one-hot matrix S_w[e,j] = w[e]*(d[e]==j)
    is built in one DVE tensor_scalar(is_equal, mult) op; PE matmul
    accumulates S_w^T @ messages into PSUM per dest block.
  - Bias is folded into the PSUM group via a rank-1 matmul
    (sqrt(deg) outer b); relu/final scale is one ACT op with
    per-partition scale=dinv.
  - deg is computed on device from a host-rectangularized w table;
    full deg is exchanged with a tiny AllGather.  u1 = dinv*(x@W1) is
    computed replicated on every core (cheap, avoids a 25MB AllGather);
    u2 = dinv*(h1@W2) is computed sharded + one AllGather of 12.8MB.
"""

import math
import os

import numpy as np

import concourse.bacc as bacc
import concourse.bass as bass
import concourse.mybir as mybir
import concourse.tile as tile
from concourse.bass_utils import run_bass_kernel_spmd

P = 128
NCORES = 8
SG = 6  # dest blocks per gather supergroup
LO_LIMIT = 32768  # int16 index reach for dma_gather

F32 = mybir.dt.float32
F16 = mybir.dt.float16
I16 = mybir.dt.int16

# toggles
U1_F16 = os.environ.get("GCN_U1_F16", "1") == "1"
DT_TAB1 = F16 if U1_F16 else F32
NP_TAB1 = np.float16 if U1_F16 else np.float32

_last_results = {}


def _wrap_idx(arr):
    """int16 stream -> [128, len/16] wrapped layout for dma_gather."""
    assert len(arr) % 16 == 0
    a = arr.reshape(-1, 16).T  # [16, len/16]
    return np.ascontiguousarray(np.tile(a, (8, 1)))  # [128, len/16]


def _prep(x, edge_index, edge_weight, W1, b1, W2, b2):
    N, F = x.shape
    H = W1.shape[1]
    C = W2.shape[1]
    assert F == 128 and H == 128 and C == 64, (F, H, C)
    E = edge_index.shape[1]

    NPC = ((N + NCORES * P - 1) // (NCORES * P)) * P
    NPAD = NPC * NCORES
    NBLK = NPC // P
    NB_ALL = NPAD // P
    HI_BASE = max(NPAD - LO_LIMIT, 0)

    src = np.asarray(edge_index[0], dtype=np.int64)
    dst = np.asarray(edge_index[1], dtype=np.int64)
    w = np.asarray(edge_weight, dtype=np.float32)
    loop = np.arange(N, dtype=np.int64)
    src = np.concatenate([src, loop])
    dst = np.concatenate([dst, loop])
    w = np.concatenate([w, np.ones(N, np.float32)])
    EA = len(src)

    blk = dst // P
    if NPAD > LO_LIMIT:
        half = (src >= LO_LIMIT).astype(np.int64)
    else:
        half = np.zeros(EA, np.int64)

    order = np.lexsort((src, half, blk))
    src, dst, w, blk, half = (
        src[order], dst[order], w[order], blk[order], half[order])

    # counts per (block, half)
    cnt = np.zeros((NB_ALL, 2), np.int64)
    np.add.at(cnt, (blk, half), 1)
    CH_LO = int(math.ceil(cnt[:, 0].max() / P))
    CH_HI = int(math.ceil(cnt[:, 1].max() / P)) if NPAD > LO_LIMIT else 0
    CH = CH_LO + CH_HI

    gid = blk * 2 + half
    gstart = np.zeros(NB_ALL * 2 + 1, np.int64)
    np.add.at(gstart[1:], gid, 1)
    gstart = np.cumsum(gstart)
    rank = np.arange(EA) - gstart[gid]

    # d/w columns: chunk-column layout [NB_ALL, CH, P]
    j_chunk = np.where(half == 0, rank // P, CH_LO + rank // P)
    slot_dw = (blk * CH + j_chunk) * P + rank % P
    d_all = np.zeros(NB_ALL * CH * P, np.float32)
    w_all = np.zeros(NB_ALL * CH * P, np.float32)
    d_all[slot_dw] = (dst % P).astype(np.float32)
    w_all[slot_dw] = w

    # gather index streams (lo / hi separately)
    lo_m = half == 0
    idx_lo_all = np.zeros(NB_ALL * CH_LO * P, np.int16)
    slot_lo = blk[lo_m] * CH_LO * P + rank[lo_m]
    idx_lo_all[slot_lo] = src[lo_m].astype(np.int16)
    if CH_HI:
        hi_m = ~lo_m
        idx_hi_all = np.zeros(NB_ALL * CH_HI * P, np.int16)
        slot_hi = blk[hi_m] * CH_HI * P + rank[hi_m]
        idx_hi_all[slot_hi] = (src[hi_m] - HI_BASE).astype(np.int16)

    # deg rectangular table: [node, slot] of incoming w (incl self loop)
    order2 = np.argsort(dst, kind="stable")
    dst2, w2 = dst[order2], w[order2]
    dcnt = np.zeros(NPAD + 1, np.int64)
    np.add.at(dcnt[1:], dst2, 1)
    S_DEG = int(dcnt.max())
    dstart = np.cumsum(dcnt)
    drank = np.arange(EA) - dstart[dst2]
    deg_rect = np.zeros((NPAD, S_DEG), np.float32)
    deg_rect[dst2, drank] = w2
    deg_rect[N:, 0] = 1.0  # pad nodes get deg=1 to avoid inf

    # per-core tensors
    xT = np.zeros((P, NPAD), NP_TAB1)
    xT[:, :N] = np.asarray(x, np.float32).T.astype(NP_TAB1)
    W1h = np.asarray(W1, np.float32).astype(NP_TAB1)
    W2h = np.asarray(W2, np.float32).astype(NP_TAB1)
    b1r = np.asarray(b1, np.float32).astype(NP_TAB1)[None, :]
    b2r = np.asarray(b2, np.float32).astype(NP_TAB1)[None, :]
    iota_row = np.tile(np.arange(P, dtype=np.float32), (P, 1))
    ident = np.eye(P, dtype=np.float32)

    common = {
        "xT": xT, "W1": W1h, "W2": W2h, "b1r": b1r, "b2r": b2r,
        "iota16": iota_row.astype(NP_TAB1),
        "ident": ident,
    }

    in_maps = []
    for i in range(NCORES):
        b0, b1_ = i * NBLK, (i + 1) * NBLK
        m = dict(common)
        m["deg_rect"] = np.ascontiguousarray(
            deg_rect[b0 * P:b1_ * P].reshape(NBLK, P, S_DEG)
            .transpose(1, 0, 2).reshape(P, NBLK * S_DEG))
        m["dcol"] = np.ascontiguousarray(
            d_all[b0 * CH * P:b1_ * CH * P].reshape(NBLK * CH, P).T)
        m["wcol"] = np.ascontiguousarray(
            w_all[b0 * CH * P:b1_ * CH * P].reshape(NBLK * CH, P).T)
        m["idx_lo"] = _wrap_idx(idx_lo_all[b0 * CH_LO * P:b1_ * CH_LO * P])
        if CH_HI:
            m["idx_hi"] = _wrap_idx(idx_hi_all[b0 * CH_HI * P:b1_ * CH_HI * P])
        in_maps.append(m)

    cfg = dict(N=N, NPC=NPC, NPAD=NPAD, NBLK=NBLK, NB_ALL=NB_ALL,
               HI_BASE=HI_BASE, CH_LO=CH_LO, CH_HI=CH_HI, CH=CH,
               S_DEG=S_DEG, H=H, C=C)
    return in_maps, cfg


_OH_CNT = [0]


def _onehot(nc, swp, io16, dcol, wcol, ndcol, nwcol, col, AL, AF):
    """S_w[e, j] = w[e] * (d[e] == j), [128,128] f16.  Roughly 1/3 of the
    builds run on the otherwise-idle ACT engine (abs + relu trick:
    relu(w - w*|iota - d|)), the rest on DVE (is_equal * w)."""
    _OH_CNT[0] += 1
    sw = swp.tile([128, 128], F16, tag="sw")
    if _OH_CNT[0] % 3 == 0:
        t1 = swp.tile([128, 128], F16, tag="oht")
        nc.scalar.activation(t1[:], io16[:], AF.Abs,
                             bias=ndcol[:, col:col + 1], scale=1.0)
        nc.scalar.activation(sw[:], t1[:], AF.Relu,
                             bias=wcol[:, col:col + 1],
                             scale=nwcol[:, col:col + 1])
    else:
        nc.vector.tensor_scalar(
            out=sw[:], in0=io16[:], scalar1=dcol[:, col:col + 1],
            scalar2=wcol[:, col:col + 1], op0=AL.is_equal, op1=AL.mult)
    return sw


def _split_gather(nc, qn, gtile, src, idx_tile, ch0, nch, elem):
    """Issue a supergroup gather as two half-gathers on different SWDGE
    queues so descriptor generation runs 2x parallel per stream."""
    h1 = (nch + 1) // 2
    for lo, hi in ((0, h1), (h1, nch)):
        if hi <= lo:
            continue
        ni = (hi - lo) * P
        nc.gpsimd.dma_gather(
            gtile[:, lo:hi, :], src,
            idx_tile[:, (ch0 + lo) * 8:(ch0 + hi) * 8],
            ni, ni, elem, single_packet=False, queue_num=qn(0))


def _build(cfg):
    NPC, NPAD, NBLK = cfg["NPC"], cfg["NPAD"], cfg["NBLK"]
    NB_ALL, HI_BASE = cfg["NB_ALL"], cfg["HI_BASE"]
    CH_LO, CH_HI, CH = cfg["CH_LO"], cfg["CH_HI"], cfg["CH"]
    S_DEG, H, C = cfg["S_DEG"], cfg["H"], cfg["C"]
    LO_ROWS = min(NPAD, LO_LIMIT)
    AX = mybir.AxisListType
    AL = mybir.AluOpType
    AF = mybir.ActivationFunctionType

    nc = bacc.Bacc("TRN2", target_bir_lowering=False, debug=False,
                   num_devices=NCORES, num_swdge_queues=4)

    xT_d = nc.dram_tensor("xT", [P, NPAD], DT_TAB1, kind="ExternalInput")
    W1_d = nc.dram_tensor("W1", [P, H], DT_TAB1, kind="ExternalInput")
    W2_d = nc.dram_tensor("W2", [P, C], DT_TAB1, kind="ExternalInput")
    b1_d = nc.dram_tensor("b1r", [1, H], DT_TAB1, kind="ExternalInput")
    b2_d = nc.dram_tensor("b2r", [1, C], DT_TAB1, kind="ExternalInput")
    io16_d = nc.dram_tensor("iota16", [P, P], DT_TAB1, kind="ExternalInput")
    id_d = nc.dram_tensor("ident", [P, P], F32, kind="ExternalInput")
    dr_d = nc.dram_tensor("deg_rect", [P, NBLK * S_DEG], F32,
                          kind="ExternalInput")
    dc_d = nc.dram_tensor("dcol", [P, NBLK * CH], F32, kind="ExternalInput")
    wc_d = nc.dram_tensor("wcol", [P, NBLK * CH], F32, kind="ExternalInput")
    il_d = nc.dram_tensor("idx_lo", [P, NBLK * CH_LO * 8], I16,
                          kind="ExternalInput")
    if CH_HI:
        ih_d = nc.dram_tensor("idx_hi", [P, NBLK * CH_HI * 8], I16,
                              kind="ExternalInput")
    out_d = nc.dram_tensor("out", [NPC, C], F32, kind="ExternalOutput")

    u1_tab = nc.dram_tensor("u1_tab", [NPAD, H], DT_TAB1)
    u2_own = nc.dram_tensor("u2_own", [NPC, H], DT_TAB1)
    u2_tab = nc.dram_tensor("u2_tab", [NPAD, H], DT_TAB1, addr_space="Shared")
    degT_own = nc.dram_tensor("degT_own", [NBLK * P], F32)
    deg_full = nc.dram_tensor("deg_full", [NPAD], F32, addr_space="Shared")
    dvr_flat = nc.dram_tensor("dvr_flat", [NBLK * P], F32)
    dva_flat = nc.dram_tensor("dva_flat", [NPAD], F32)

    rg = [list(range(NCORES))]
    sgroups = [list(range(s, min(s + SG, NBLK))) for s in range(0, NBLK, SG)]
    _q = [0]

    def qn(_):
        _q[0] = (_q[0] + 1) % 4
        return _q[0]
    STAGE = int(os.environ.get("GCN_STAGE", "4"))

    with tile.TileContext(nc) as tc:
        with (
            tc.tile_pool(name="const", bufs=1) as cp,
            tc.tile_pool(name="work", bufs=2) as wp,
            tc.tile_pool(name="sw", bufs=4) as swp,
            tc.tile_pool(name="psum", bufs=2, space="PSUM") as pp,
        ):
            # ---- constants ----
            W1s = cp.tile([P, H], DT_TAB1)
            nc.sync.dma_start(W1s[:], W1_d[:, :])
            W2s = cp.tile([P, C], DT_TAB1)
            nc.sync.dma_start(W2s[:], W2_d[:, :])
            b1s = cp.tile([1, H], DT_TAB1)
            nc.sync.dma_start(b1s[:], b1_d[:, :])
            b2s = cp.tile([1, C], DT_TAB1)
            nc.sync.dma_start(b2s[:], b2_d[:, :])
            io16 = cp.tile([P, P], DT_TAB1)
            nc.sync.dma_start(io16[:], io16_d[:, :])
            idn = cp.tile([P, P], F32)
            nc.sync.dma_start(idn[:], id_d[:, :])
            dcol = cp.tile([P, NBLK * CH], F32)
            nc.sync.dma_start(dcol[:], dc_d[:, :])
            wcol = cp.tile([P, NBLK * CH], F32)
            nc.sync.dma_start(wcol[:], wc_d[:, :])
            ilo = cp.tile([P, NBLK * CH_LO * 8], I16)
            nc.sync.dma_start(ilo[:], il_d[:, :])
            if CH_HI:
                ihi = cp.tile([P, NBLK * CH_HI * 8], I16)
                nc.sync.dma_start(ihi[:], ih_d[:, :])
            drect = cp.tile([P, NBLK * S_DEG], F32)
            nc.sync.dma_start(drect[:], dr_d[:, :])
            ndcol = cp.tile([P, NBLK * CH], F32)
            nc.vector.tensor_scalar(out=ndcol[:], in0=dcol[:], scalar1=-1.0,
                                    scalar2=None, op0=AL.mult)
            nwcol = cp.tile([P, NBLK * CH], F32)
            nc.vector.tensor_scalar(out=nwcol[:], in0=wcol[:], scalar1=-1.0,
                                    scalar2=None, op0=AL.mult)

            # ---- deg / dinv (local rows) ----
            deg = cp.tile([P, NBLK], F32)
            nc.vector.tensor_reduce(
                deg[:], drect[:].rearrange("p (b s) -> p b s", s=S_DEG),
                axis=AX.X, op=AL.add)
            rec = cp.tile([P, NBLK], F32)
            nc.vector.reciprocal(rec[:], deg[:])
            dinv = cp.tile([P, NBLK], F32)
            nc.scalar.sqrt(dinv[:], rec[:])  # dinv = 1/sqrt(deg)
            dinvr = cp.tile([P, NBLK], F32)
            nc.scalar.sqrt(dinvr[:], deg[:])  # sqrt(deg) = 1/dinv
            dinv2 = cp.tile([P, NBLK], F32)
            nc.vector.tensor_tensor(out=dinv2[:], in0=dinv[:], in1=dinv[:],
                                    op=AL.mult)
            # transposed copy of dinvr, round-tripped through DRAM into a
            # single-partition row so rank-1 bias matmuls can slice it along
            # the free dim (matmul lhsT needs partition base 0).
            pt = pp.tile([P, P], F32, tag="ptr")
            nc.tensor.transpose(pt[:NBLK, :], dinvr[:], idn[:])
            dinvrT = cp.tile([NBLK, P], F32)
            nc.vector.tensor_copy(dinvrT[:], pt[:NBLK, :])
            nc.sync.dma_start(
                dvr_flat.ap().rearrange("(b p) -> b p", p=P), dinvrT[:])
            dvr32 = cp.tile([1, NBLK * P], F32)
            nc.sync.dma_start(dvr32[:], dvr_flat.ap()[None, :])
            dvr16 = cp.tile([1, NBLK * P], DT_TAB1)
            nc.vector.tensor_copy(dvr16[:], dvr32[:])
            # deg -> DRAM (block,p order) -> AllGather
            pt2 = pp.tile([P, P], F32, tag="ptr")
            nc.tensor.transpose(pt2[:NBLK, :], deg[:], idn[:])
            degT = cp.tile([NBLK, P], F32)
            nc.vector.tensor_copy(degT[:], pt2[:NBLK, :])
            nc.sync.dma_start(
                degT_own.ap().rearrange("(b p) -> b p", p=P), degT[:])
            nc.gpsimd.collective_compute(
                "AllGather", AL.bypass, replica_groups=rg,
                ins=[degT_own.ap()], outs=[deg_full.ap()])
            # load deg_full -> [P, NB_ALL] (via transposes), compute dinv_all
            dega = cp.tile([P, NB_ALL], F32)
            degf2d = deg_full.ap().rearrange("(b p) -> b p", p=P)
            for t0 in range(0, NB_ALL, P):
                tb = min(P, NB_ALL - t0)
                dl = wp.tile([P, P], F32, tag="degload")
                nc.sync.dma_start(dl[:tb, :], degf2d[t0:t0 + tb, :])
                ptd = pp.tile([P, P], F32, tag="ptr")
                nc.tensor.transpose(ptd[:, :tb], dl[:tb, :], idn[:tb, :tb])
                nc.vector.tensor_copy(dega[:, t0:t0 + tb], ptd[:, :tb])
            reca = cp.tile([P, NB_ALL], F32)
            nc.vector.reciprocal(reca[:], dega[:])
            dinva = cp.tile([P, NB_ALL], F32)
            nc.scalar.sqrt(dinva[:], reca[:])

            tc.strict_bb_all_engine_barrier()

            if STAGE < 2:
                return _finish(nc)
            # ---- u1 = dinv * (x @ W1), full table, replicated ----
            # stream xT in wide tiles; 4 blocks share one PSUM bank; the
            # PSUM drain (scale by dinv + cast) runs per block on DVE.
            XW = 16  # blocks per stream tile
            for T0 in range(0, NB_ALL, XW):
                tb = min(XW, NB_ALL - T0)
                xs = wp.tile([P, XW * P], DT_TAB1, tag="xstream", bufs=2)
                nc.sync.dma_start(xs[:, :tb * P],
                                  xT_d[:, T0 * P:(T0 + tb) * P])
                for q0 in range(0, tb, 4):
                    qb = min(4, tb - q0)
                    pu = pp.tile([P, 4 * H], F32, tag="acc")
                    u1b = wp.tile([P, 4 * H], DT_TAB1, tag="u1b", bufs=3)
                    for k in range(qb):
                        B = T0 + q0 + k
                        nc.tensor.matmul(
                            pu[:, k * H:(k + 1) * H],
                            xs[:, (q0 + k) * P:(q0 + k + 1) * P],
                            W1s[:], start=True, stop=True)
                        nc.vector.tensor_scalar(
                            out=u1b[:, k * H:(k + 1) * H],
                            in0=pu[:, k * H:(k + 1) * H],
                            scalar1=dinva[:, B:B + 1],
                            scalar2=None, op0=AL.mult)
                    B0 = T0 + q0
                    nc.sync.dma_start(
                        u1_tab.ap().rearrange("(B p) f -> p B f", p=P)
                        [:, B0:B0 + qb, :],
                        u1b[:].rearrange("p (B f) -> p B f", f=H)[:, :qb, :])

            if STAGE < 3:
                return _finish(nc)
            # ---- layer 1 message pass + u2 ----
            u1_lo = u1_tab[0:LO_ROWS, :]
            u1_hi = u1_tab[HI_BASE:NPAD, :] if CH_HI else None
            for blist in sgroups:
                nsg = len(blist)
                b0 = blist[0]
                glo = wp.tile([P, nsg * CH_LO, H], DT_TAB1, tag="glo", bufs=3)
                _split_gather(nc, qn, glo, u1_lo, ilo, b0 * CH_LO, nsg * CH_LO, H)
                if CH_HI:
                    ghi = wp.tile([P, nsg * CH_HI, H], DT_TAB1, tag="ghi",
                                  bufs=3)
                    _split_gather(nc, qn, ghi, u1_hi, ihi, b0 * CH_HI,
                                  nsg * CH_HI, H)
                for b in blist:
                    # flipped scatter: accumulate h1^T = [feat, dest] in PSUM
                    ph = pp.tile([P, P], F32, tag="acc")
                    for j in range(CH):
                        col = b * CH + j
                        sw = _onehot(nc, swp, io16, dcol, wcol, ndcol, nwcol,
                                     col, AL, AF)
                        if j < CH_LO:
                            lhs = glo[:, (b - b0) * CH_LO + j, :]
                        else:
                            lhs = ghi[:, (b - b0) * CH_HI + (j - CH_LO), :]
                        nc.tensor.matmul(ph[:], lhs, sw[:],
                                         start=(j == 0), stop=False)
                    nc.tensor.matmul(ph[:], b1s[:], dvr16[:, b * P:(b + 1) * P],
                                     start=False, stop=True)
                    # relu(dinv*t) = dinv*relu(t): defer both dinv factors to
                    # the u2 drain (dinv^2); pure relu here.
                    h1T = wp.tile([P, P], DT_TAB1, tag="h1T")
                    nc.vector.tensor_scalar(
                        out=h1T[:], in0=ph[:], scalar1=0.0, scalar2=None,
                        op0=AL.max)
                    pu2 = pp.tile([P, C], F32, tag="accC")
                    nc.tensor.matmul(pu2[:], h1T[:], W2s[:],
                                     start=True, stop=True)
                    u2b = wp.tile([P, H], DT_TAB1, tag="u2b")
                    nc.vector.memset(u2b[:, C:], 0)
                    nc.vector.tensor_scalar(
                        out=u2b[:, :C], in0=pu2[:], scalar1=dinv2[:, b:b + 1],
                        scalar2=None, op0=AL.mult)
                    nc.sync.dma_start(u2_own[b * P:(b + 1) * P, :], u2b[:])

            if STAGE < 4:
                return _finish(nc)
            nc.gpsimd.collective_compute(
                "AllGather", AL.bypass, replica_groups=rg,
                ins=[u2_own.ap()], outs=[u2_tab.ap()])

            # ---- layer 2 message pass ----
            u2_lo = u2_tab[0:LO_ROWS, :]
            u2_hi = u2_tab[HI_BASE:NPAD, :] if CH_HI else None
            for blist in sgroups:
                nsg = len(blist)
                b0 = blist[0]
                glo = wp.tile([P, nsg * CH_LO, H], DT_TAB1, tag="glo", bufs=3)
                _split_gather(nc, qn, glo, u2_lo, ilo, b0 * CH_LO, nsg * CH_LO, H)
                if CH_HI:
                    ghi = wp.tile([P, nsg * CH_HI, H], DT_TAB1, tag="ghi", bufs=3)
                    _split_gather(nc, qn, ghi, u2_hi, ihi, b0 * CH_HI,
                                  nsg * CH_HI, H)
                for b in blist:
                    po = pp.tile([P, C], F32, tag="accC")
                    for j in range(CH):
                        col = b * CH + j
                        sw = _onehot(nc, swp, io16, dcol, wcol, ndcol, nwcol,
                                     col, AL, AF)
                        if j < CH_LO:
                            rhs = glo[:, (b - b0) * CH_LO + j, :C]
                        else:
                            rhs = ghi[:, (b - b0) * CH_HI + (j - CH_LO), :C]
                        nc.tensor.matmul(po[:], sw[:], rhs,
                                         start=(j == 0), stop=False)
                    nc.tensor.matmul(po[:], dvr16[:, b * P:(b + 1) * P], b2s[:],
                                     start=False, stop=True)
                    ob = wp.tile([P, C], F32, tag="ob")
                    nc.scalar.activation(ob[:], po[:], AF.Copy,
                                         bias=0.0, scale=dinv[:, b:b + 1])
                    nc.sync.dma_start(out_d[b * P:(b + 1) * P, :], ob[:])
    return _finish(nc)


def _finish(nc):
    nc.compile()
    return nc


def kernel(x, edge_index, edge_weight, W1, b1, W2, b2):
    in_maps, cfg = _prep(x, edge_index, edge_weight, W1, b1, W2, b2)
    nc = _build(cfg)
    trace = os.environ.get("GCN_TRACE", "0") == "1"
    res = run_bass_kernel_spmd(nc, in_maps, core_ids=list(range(NCORES)),
                               trace=trace)
    _last_results["exec_time_ns"] = res.exec_time_ns
    _last_results["results"] = res
    out = np.concatenate([r["out"] for r in res.results], axis=0)
    return np.ascontiguousarray(out[:cfg["N"]])
